# revision 7
# baseline (speedup 1.0000x reference)
"""Trainium2 Bass kernel for nn_AttentionModule_66537633349985 (segment attention pooling).

Math (per graph b): out[b] = sum_n attn_n * emb_n over nodes n with batch[n]==b,
where attn = softmax_b(w_a . tanh(W_c @ emb + b_c) + b_a). The +b_a and the
segment-max subtraction cancel in the softmax ratio, so neither is computed
(scores are bounded by sum|w_a| <= ~11, so exp never overflows in f32).

Sharding: nodes are split evenly across the 8 cores (125000 each, zero-padded
to 62 macro-tiles of 2048). All on-chip work is in H-on-partitions layout
[128, nodes]; the host pre-transposes embeddings once.

Device, per 2048-node macro-tile:
    t  = W_c @ embT              fp32r matmul, W stationary (4 x N=512)
    tT = tanh(t + b_c)           ACT, PSUM -> SBUF (f32r)
    s  = w_a . tT                matmul -> [1,512] rows at partitions 0/32/64/96
    e  = exp(s)                  ACT over the whole PSUM bank (junk rows unused)
    eb = ones (x) e_row          PE rank-1 broadcast to 128 partitions (f32r)
    P[:, blk] = sum(embT * eb)   fused DVE tensor_tensor_reduce per 512-chunk
    dump P [128, 4] and e [4, 512] to HBM.

Host epilogue: per-segment sums over whole 512-node blocks from P; blocks
containing a segment boundary are recomputed exactly on the host from emb and
the dumped exp(scores); denominators via bincount of dumped exp(scores);
divide and assemble the [1024, 128] output.
"""
import numpy as np

import concourse.bass as bass
import concourse.bacc as bacc
import concourse.tile as tile
import concourse.mybir as mybir
from concourse.bass_utils import run_bass_kernel_spmd

H = 128            # hidden dim
B = 1024           # number of graphs
NCORES = 8
TM = 2048          # nodes per macro-tile
NCH = TM // 512    # 512-node chunks per macro-tile
BLK = 512          # block size of the on-chip partial sums

f32 = mybir.dt.float32
f32r = mybir.dt.float32r
bf16 = mybir.dt.bfloat16

_BUILD_CACHE: dict = {}


def build_bass(L: int, repeat: int = 1) -> "bacc.Bacc":
    """Per-core Bass program for an [H, L] H-major embedding shard.

    repeat > 1 replays the whole pipeline (for marginal-time measurement);
    outputs are simply rewritten identically."""
    key = (L, repeat)
    if key in _BUILD_CACHE:
        return _BUILD_CACHE[key]
    assert L % TM == 0
    nmacro = L // TM

    nc = bacc.Bacc("TRN2", target_bir_lowering=False, debug=False)

    embT_d = nc.dram_tensor("embT", [H, L], f32r, kind="ExternalInput")
    W_d = nc.dram_tensor("W", [H, H], f32r, kind="ExternalInput")      # holds W_c.T
    wa_d = nc.dram_tensor("wa", [H, 1], f32, kind="ExternalInput")
    bc_d = nc.dram_tensor("bc", [H, 1], f32, kind="ExternalInput")
    P_d = nc.dram_tensor("P", [nmacro, H, NCH], f32, kind="ExternalOutput")
    e_d = nc.dram_tensor("e", [nmacro, NCH, 512], f32, kind="ExternalOutput")

    Tanh = mybir.ActivationFunctionType.Tanh
    Exp = mybir.ActivationFunctionType.Exp

    with tile.TileContext(nc) as tc:
        with (
            tc.tile_pool(name="const", bufs=1) as cpool,
            tc.tile_pool(name="sbuf", bufs=3) as pool,
            tc.tile_pool(name="pt", bufs=2, space="PSUM") as pt_pool,
            tc.tile_pool(name="ps", bufs=2, space="PSUM") as ps_pool,
            tc.tile_pool(name="pe", bufs=2, space="PSUM") as pe_pool,
        ):
            W_sb = cpool.tile([H, H], f32r)
            wa_sb = cpool.tile([H, 1], f32)
            wa_bf = cpool.tile([H, 1], bf16)
            bc_sb = cpool.tile([H, 1], f32)
            ones_sb = cpool.tile([H, H], f32r)
            nc.sync.dma_start(W_sb[:], W_d[:])
            nc.sync.dma_start(wa_sb[:], wa_d[:])
            nc.sync.dma_start(bc_sb[:], bc_d[:])
            nc.vector.tensor_copy(wa_bf[:], wa_sb[:])
            nc.vector.memset(ones_sb[:].bitcast(f32), 1.0)

            for m in [mm for _ in range(repeat) for mm in range(nmacro)]:
                emb_sb = pool.tile([H, TM], f32r, tag="emb")
                nc.sync.dma_start(emb_sb[:], embT_d[:, m * TM:(m + 1) * TM])

                # t = W_c @ embT (fp32r); two half-tiles, each its own
                # 2-bank PSUM buffer so tanh(m) overlaps mm1(m+1)
                tT_sb = pool.tile([H, TM], bf16, tag="tT")
                for h in range(2):
                    psum_t = pt_pool.tile([H, TM // 2], f32, tag="pt")
                    for j in range(2):
                        nc.tensor.matmul(
                            psum_t[:, j * 512:(j + 1) * 512],
                            W_sb[:],
                            emb_sb[:, (2 * h + j) * 512:(2 * h + j + 1) * 512],
                            start=True, stop=True,
                        )
                    nc.scalar.activation(
                        tT_sb[:, h * 1024:(h + 1) * 1024], psum_t[:],
                        Tanh, bias=bc_sb[:])

                # s = w_a . tT -> [1, 512] rows at partitions 0/32/64/96
                psum_s = ps_pool.tile([H, 512], f32, tag="ps")
                for j in range(NCH):
                    nc.tensor.matmul(
                        psum_s[32 * j:32 * j + 1, :],
                        wa_bf[:],
                        tT_sb[:, j * 512:(j + 1) * 512],
                        start=True, stop=True,
                        tile_position=(0, 32 * j),
                    )
                e_sb = pool.tile([H, 512], f32r, tag="e")
                nc.scalar.activation(e_sb[:], psum_s[:], Exp)
                nc.sync.dma_start(e_d[m], e_sb[0:H:32, :].bitcast(f32))

                # wE = embT * exp(s): PE rank-1 broadcast of the e row,
                # then multiply on DVE; block sums via tensor_reduce.
                P_sb = pool.tile([H, NCH], f32, tag="P")
                wE_sb = pool.tile([H, TM], f32, tag="wE")
                for j in range(NCH):
                    psum_eb = pe_pool.tile([H, 512], f32, tag="pe")
                    nc.tensor.matmul(
                        psum_eb[:],
                        ones_sb[32 * j:32 * j + 1, :],
                        e_sb[32 * j:32 * j + 1, :],
                        start=True, stop=True,
                        tile_position=(32 * j, 0),
                    )
                    nc.vector.tensor_tensor(
                        out=wE_sb[:, j * 512:(j + 1) * 512],
                        in0=emb_sb[:, j * 512:(j + 1) * 512].bitcast(f32),
                        in1=psum_eb[:],
                        op=mybir.AluOpType.mult,
                    )
                # block sums: chunks 0-2 on DVE, chunk 3 on ACT (accum copy)
                nc.vector.tensor_reduce(
                    out=P_sb[:, 0:3],
                    in_=wE_sb[:, 0:3 * 512].rearrange("p (b k) -> p b k", k=BLK),
                    axis=mybir.AxisListType.X,
                    op=mybir.AluOpType.add,
                )
                scr_sb = pool.tile([H, 512], f32, tag="scr")
                nc.scalar.activation(
                    scr_sb[:], wE_sb[:, 3 * 512:4 * 512],
                    mybir.ActivationFunctionType.Copy,
                    accum_out=P_sb[:, 3:4],
                )
                nc.sync.dma_start(P_d[m], P_sb[:])

    nc.compile()
    _BUILD_CACHE[key] = nc
    return nc


def kernel(**inputs) -> np.ndarray:
    emb = np.ascontiguousarray(np.asarray(inputs["embeddings"], dtype=np.float32))
    batch = np.asarray(inputs["batch"]).astype(np.int64)
    W_c = np.asarray(inputs["W_c"], dtype=np.float32)
    b_c = np.asarray(inputs["b_c"], dtype=np.float32)
    w_a = np.asarray(inputs["w_a"], dtype=np.float32)
    # b_a cancels in the softmax; unused.

    N = emb.shape[0]
    assert N % NCORES == 0
    SH = N // NCORES                      # nodes per core
    L = ((SH + TM - 1) // TM) * TM        # zero-padded shard length

    embT = np.zeros((NCORES, H, L), dtype=np.float32)
    for c in range(NCORES):
        embT[c][:, :SH] = emb[c * SH:(c + 1) * SH].T

    nc = build_bass(L)
    Wt = np.ascontiguousarray(W_c.T)
    wa_col = np.ascontiguousarray(w_a[:, None])
    bc_col = np.ascontiguousarray(b_c[:, None])
    in_maps = [
        {"embT": embT[c], "W": Wt, "wa": wa_col, "bc": bc_col}
        for c in range(NCORES)
    ]
    res = run_bass_kernel_spmd(nc, in_maps, core_ids=list(range(NCORES)))

    num = np.zeros((B, H), dtype=np.float64)
    e_global = np.empty(N, dtype=np.float32)
    nblk_real = (SH + BLK - 1) // BLK
    for c in range(NCORES):
        P = res.results[c]["P"]                          # [nmacro, H, NCH]
        e_flat = res.results[c]["e"].reshape(-1)         # [L]
        e_global[c * SH:(c + 1) * SH] = e_flat[:SH]
        P_flat = np.moveaxis(P, 1, 0).reshape(H, -1)     # [H, L//BLK]
        for b in range(nblk_real):
            g0 = c * SH + BLK * b
            g1 = min(g0 + BLK, (c + 1) * SH)
            s0 = batch[g0]
            s1 = batch[g1 - 1]
            if s0 == s1:
                num[s0] += P_flat[:, b]
            else:
                # boundary block: recompute exactly on host per segment run
                segs = batch[g0:g1]
                eb = e_flat[BLK * b: BLK * b + (g1 - g0)].astype(np.float64)
                cuts = np.concatenate(
                    [[0], np.flatnonzero(np.diff(segs)) + 1, [g1 - g0]])
                for r in range(len(cuts) - 1):
                    r0, r1 = cuts[r], cuts[r + 1]
                    num[segs[r0]] += eb[r0:r1] @ emb[g0 + r0: g0 + r1]
    den = np.bincount(batch, weights=e_global, minlength=B)
    return (num / den[:, None]).astype(np.float32)
